# revision 23
# baseline (speedup 1.0000x reference)
"""Trainium2 Bass kernel for n-iteration Jacobi (3x3 cross stencil, reflect pad).

x_{t+1} = 0.25*(V + H) x_t + f,  f = COF*layout (|f| ~ 2.4e-9, contributes
< 3e-6 relative to the output; dropped).

V (vertical) and H (horizontal) neighbor-sum operators with this reflect
boundary are exactly diagonalized by the DCT-I basis v_k[i] = cos(pi*i*k/1023),
eigenvalues lam_k = 2*cos(pi*k/1023).  n Jacobi iterations collapse to one
spectral sandwich per image:

    out = C_k @ (Lam2D * (Cinv_k @ X @ Cinv_k^T)) @ C_k^T
    Lam2D[a,b] = ((lam_a + lam_b)/4)^n

Reductions on top of the plain sandwich:
  1. Mode truncation: keep K=512 of 1024 modes per axis for n=50 (256 lowest
     + 256 highest; max truncated |Lam| ~ 4e-4).
  2. Even/odd folding: cos(pi*k*(1023-i)/1023) = (-1)^k cos(pi*k*i/1023),
     so folding the spatial axes into symmetric/antisymmetric halves halves
     every contraction.  Input fold on the host; output parity recombination
     ALSO on the host (kernel emits the even/odd partial sums Oe/Oo).
  3. Corner sparsity: Lam2D is non-negligible only for same-corner mode
     pairs (low-low near (0,0), high-high near (pi,pi)); cross terms are
     <= 0.147^n ~ 0 for n>=30.  The mode-space passes (C: forward-horizontal
     + Lam, E: inverse-horizontal) contract only same-corner blocks, halving
     both.  Enabled by a corner-major mode layout in UT (scatter copy-out
     from passA's PSUM).
  4. No PE transposes: passes needing transposed outputs run with the data
     as the stationary lhsT operand.
  5. DMA: the two HWDGE rings (sync, act) carry the startup-critical bytes
     in priority order so passA can start as soon as ~1.5 MiB has landed.

Per image: 120 matmuls (A:64x256c, C:16x256c, E:8x512c, G:32x512c) ~ 41K PE
rows at 1 row/cycle.  All matmul operands fp16 (PSUM accumulates fp32).
Per core: 2 of 16 images, passes software-pipelined across the two images.
"""

import math
from contextlib import ExitStack

import numpy as np

NX = 1024
N_CORES = 8
IMGS_PER_CORE = 2
LN_TAU = math.log(1e4)

_compiled_cache = {}


def _choose_K(n_iter):
    # keep modes with ((lam_a+lam_b)/4)^n >= 1e-4; parity folding needs
    # K to be a multiple of 256
    R = int(math.ceil(1023.0 / math.pi * math.sqrt(2.0 * LN_TAU / max(n_iter, 1))))
    K = min(1024, ((2 * R + 255) // 256) * 256)
    return K


def _dct_mats():
    i = np.arange(NX)
    C = np.cos(np.pi * np.outer(i, i) / (NX - 1))
    lam = 2.0 * np.cos(np.pi * i / (NX - 1))
    w = np.ones(NX)
    w[0] = w[-1] = 0.5
    s = math.sqrt(2.0 / (NX - 1))
    # C^{-1} = (2/(N-1)) W C W; balance fp16 range: A1 = Cinv/s, B1 = C*s
    A1 = (2.0 / (NX - 1) / s) * (w[:, None] * C * w[None, :])
    B1 = C * s
    return A1, B1, lam


def _kperm(K):
    R = K // 2
    kept = np.r_[0:R, NX - R:NX]
    return np.r_[kept[kept % 2 == 0], kept[kept % 2 == 1]]  # evens, then odds


# ---------------------------------------------------------------- fast path
# K=512 only.  kperm blocks (128 modes each): 0=low-even, 1=high-even,
# 2=low-odd, 3=high-odd.  corner(block b) = b % 2 (0=low, 1=high).

def _host_weights_k512(n_iter):
    K = 512
    A1, B1, lam = _dct_mats()
    kperm = _kperm(K)
    A1t = A1[kperm, :512].T               # [512 (i'/j'), K]   fwd weights
    B1t = B1[:512, :][:, kperm].T         # [K, 512 (j'/i')]   inv weights
    # WA par-major: WA[c, par*1024 + ci*256 + m] = A1t[ci*128 + c,
    # par*256 + m] so the par-0 half is a 0.25 MiB startup-critical DMA
    WA0 = A1t.reshape(4, 128, K).transpose(1, 0, 2).reshape(128, 4, 2, 256)
    WA = np.ascontiguousarray(
        WA0.transpose(0, 2, 1, 3).reshape(128, 4 * K)).astype(np.float16)
    # WB[c, kblk*512 + f] = B1t[kblk*128 + c, f]  (kblk: mode block)
    WB = B1t.reshape(4, 128, 512).transpose(1, 0, 2)
    WB = np.ascontiguousarray(WB.reshape(128, 4 * 512)).astype(np.float16)
    # LAM2[c, ko*256 + q*128 + m] = Lam(kperm[ko*128+c], kperm[q*256 +
    # corner(ko)*128 + m]) -- same-corner (kh, kv) pairs only, kv parity q
    Lam = ((lam[kperm][:, None] + lam[kperm][None, :]) / 4.0) ** n_iter
    LAM = np.empty((128, 4 * 256), np.float32)
    for ko in range(4):
        cor = ko % 2
        for q in range(2):
            kv_cols = np.arange(q * 256 + cor * 128, q * 256 + cor * 128 + 128)
            LAM[:, ko * 256 + q * 128: ko * 256 + (q + 1) * 128] = (
                Lam[ko * 128:(ko + 1) * 128][:, kv_cols])
    return {"wa": WA, "wb": WB, "lam": LAM}


def _build_program_k512(n_iter):
    import concourse.bacc as bacc
    import concourse.mybir as mybir
    import concourse.tile as tile

    K = 512
    f16 = mybir.dt.float16
    f32 = mybir.dt.float32
    mult = mybir.AluOpType.mult
    add = mybir.AluOpType.add
    sub = mybir.AluOpType.subtract

    nc = bacc.Bacc("TRN2", target_bir_lowering=False, debug=False)
    # x0: per image the exact SBUF layout [128, 16*512] (quadrant pos, block
    # ci at cols (pos*4+ci)*512); shape-preserving DMAs only
    x0_d = nc.dram_tensor("x0", [IMGS_PER_CORE * 128, 16 * 512], f16,
                          kind="ExternalInput").ap()
    wa_d = nc.dram_tensor("wa", [128, 4 * K], f16, kind="ExternalInput").ap()
    wb_d = nc.dram_tensor("wb", [128, 4 * 512], f16,
                          kind="ExternalInput").ap()
    lam_d = nc.dram_tensor("lam", [128, 4 * 256], f32, kind="ExternalInput").ap()
    # y: per (img, io, hf) strip of 128 rows: [Oe(512) | Oo(512)]; host
    # recombines parities and unfolds
    y_d = nc.dram_tensor("y", [IMGS_PER_CORE * 8 * 128, NX], f16,
                         kind="ExternalOutput").ap()

    with tile.TileContext(nc) as tc, ExitStack() as ctx:
        wp = ctx.enter_context(tc.tile_pool(name="w", bufs=1))
        bp = ctx.enter_context(tc.tile_pool(name="b", bufs=1))
        pmm = ctx.enter_context(tc.tile_pool(name="pmm", bufs=8, space="PSUM"))
        sp = ctx.enter_context(tc.tile_pool(name="sp", bufs=6))

        WA = wp.tile([128, 4 * K], f16)
        WB = wp.tile([128, 4 * 512], f16)
        LAM = wp.tile([128, 4 * 256], f32)

        # Xq: 16 blocks of [128, 512]: pos = 2*pj + par (pj: j-fold parity,
        # par: i-fold parity), block = pos*4 + ci
        Xq = [bp.tile([128, 16 * 512], f16, name=f"x{s}") for s in range(2)]
        # UT_pj[j', kv], corner-major within each jb block of 512:
        # col = jb*512 + corner*256 + par*128 + c
        UTe = [bp.tile([128, 4 * K], f16, name=f"ute{s}") for s in range(2)]
        UTo = [bp.tile([128, 4 * K], f16, name=f"uto{s}") for s in range(2)]
        # WC[kh, kv]*Lam: col = ko*256 + kvpar*128 + m (kv same corner as ko)
        WC = [bp.tile([128, 4 * 256], f16, name=f"wc{s}") for s in range(2)]
        # ZT[kv, col]: strip kvb at col kvb*1024: [sym j' 512 | anti j' 512]
        ZT = [bp.tile([128, 4 * NX], f16, name=f"zt{s}") for s in range(2)]
        # O[(io*2+hf)*1024 + [Oe 512 | Oo 512]] -- even/odd kv partial sums
        Ot = [bp.tile([128, 8 * NX // 1], f16, name=f"ot{s}") for s in range(2)]

        # PE warmup: ramp the tensor engine's pstate on zeros while the
        # first input/weight DMAs are still in flight
        Wz = bp.tile([128, 512], f16, name="wz")
        nc.gpsimd.memset(Wz[:], 0.0)
        pw = pmm.tile([128, 512], f32, name="pw", tag="mm")
        for r in range(8):
            nc.tensor.matmul(pw[:], Wz[:, :128], Wz[:],
                             start=(r == 0), stop=(r == 7))
        # fine-grained filler so the queue can drain the moment data lands
        pw2 = pmm.tile([128, 512], f32, name="pw2", tag="mm")
        for r in range(8):
            nc.tensor.matmul(pw2[:, :128], Wz[:, :128], Wz[:, :128],
                             start=(r == 0), stop=(r == 7))

        def passA_pj(s, pj):
            # UT_pj[j', k] = sum_{i'} Xq[pos][i', j'] * A1t[i', k]
            # lhsT = input quadrant block, rhs = WA par-slice; the PSUM is
            # par-major [par: low|high]; the copy-out scatters to the
            # corner-major UT layout [cor: par0|par1].  All par-0 groups are
            # emitted before any par-1 group so the PE consumes data in DMA
            # arrival order.
            dst = UTe[s] if pj == 0 else UTo[s]
            ptiles = []
            for jb in range(4):
                p = pmm.tile([128, 512], f32, name="pm", tag="mm")
                ptiles.append(p)
                pos = 2 * pj
                for ci in range(4):
                    nc.tensor.matmul(
                        p[:, 0:256],
                        Xq[s][:, (pos * 4 + ci) * 512 + jb * 128:
                               (pos * 4 + ci) * 512 + jb * 128 + 128],
                        WA[:, ci * 256: (ci + 1) * 256],
                        start=(ci == 0), stop=(ci == 3))
            for jb in range(4):
                p = ptiles[jb]
                pos = 2 * pj + 1
                for ci in range(4):
                    nc.tensor.matmul(
                        p[:, 256:512],
                        Xq[s][:, (pos * 4 + ci) * 512 + jb * 128:
                               (pos * 4 + ci) * 512 + jb * 128 + 128],
                        WA[:, 1024 + ci * 256: 1024 + (ci + 1) * 256],
                        start=(ci == 0), stop=(ci == 3))
                src = p[:].rearrange("p (par cor c) -> p par cor c",
                                     par=2, cor=2, c=128)
                out = dst[:, jb * 512:(jb + 1) * 512].rearrange(
                    "p (cor par c) -> p par cor c", cor=2, par=2, c=128)
                if (pj * 4 + jb) % 2 == 0:
                    nc.scalar.copy(out, src)
                else:
                    nc.vector.tensor_scalar_mul(out, src, 1.0)

        def passC_kg(s, kg):
            # WC[kh, kv] = Lam * sum_{j'} A1p[kh, j'] UT_{par(kh)}[j', kv]
            # same-corner kv only (cross-corner Lam^n ~ 0).  kg=0 (even kh)
            # depends only on UTe (pj=0), kg=1 only on UTo -- interleaved
            # between passA pj-phases to absorb input-DMA arrival gaps.
            p = pmm.tile([128, 512], f32, name="pm", tag="mm")
            for half in range(2):
                ko = kg * 2 + half          # kperm block: 0=LE 1=HE 2=LO 3=HO
                cor = ko % 2
                kpar = ko // 2              # WA par-major half
                rhs_src = UTe[s] if ko < 2 else UTo[s]
                for jb in range(4):
                    nc.tensor.matmul(
                        p[:, half * 256:(half + 1) * 256],
                        WA[:, kpar * 1024 + jb * 256 + cor * 128:
                            kpar * 1024 + jb * 256 + cor * 128 + 128],
                        rhs_src[:, jb * 512 + cor * 256:
                                jb * 512 + cor * 256 + 256],
                        start=(jb == 0), stop=(jb == 3))
            for h in range(2):          # halves so passE's first consumer
                c0 = kg * 512 + h * 256  # chunk unblocks ~0.35us earlier
                nc.vector.tensor_tensor(
                    WC[s][:, c0:c0 + 256], p[:, h * 256:(h + 1) * 256],
                    LAM[:, c0:c0 + 256], op=mult)

        def passE(s):
            # ZeT/ZoT[kv, j'] = sum_{kh even/odd, same corner} WC[kh, kv]
            #                   * B1t[kh, j']
            # ZT strip: sym = Ze+Zo (Z at j'), anti = Ze-Zo (Z at 1023-j')
            for kvb in range(4):            # kv block: 0=LE 1=HE 2=LO 3=HO
                cor = kvb % 2
                kvpar = kvb // 2            # 0 = even kv chunk, 1 = odd
                ko_e = cor                  # even-kh block, same corner
                ko_o = 2 + cor              # odd-kh block, same corner
                pe = pmm.tile([128, 512], f32, name="pe", tag="mm")
                po = pmm.tile([128, 512], f32, name="po", tag="mm")
                nc.tensor.matmul(
                    pe[:], WC[s][:, ko_e * 256 + kvpar * 128:
                                 ko_e * 256 + kvpar * 128 + 128],
                    WB[:, ko_e * 512:(ko_e + 1) * 512], start=True, stop=True)
                nc.tensor.matmul(
                    po[:], WC[s][:, ko_o * 256 + kvpar * 128:
                                 ko_o * 256 + kvpar * 128 + 128],
                    WB[:, ko_o * 512:(ko_o + 1) * 512], start=True, stop=True)
                ps = sp.tile([128, 512], f16, name="ps", tag="ps")
                nc.scalar.mul(ps[:], po[:], 2.0)
                nc.vector.scalar_tensor_tensor(
                    ZT[s][:, kvb * NX: kvb * NX + 512], ps[:], 0.5, pe[:],
                    op0=mult, op1=add)
                nc.gpsimd.tensor_tensor(
                    ZT[s][:, kvb * NX + 512: (kvb + 1) * NX],
                    ZT[s][:, kvb * NX: kvb * NX + 512], ps[:], op=sub)

        def passG(s, img):
            # Oe/Oo[i', col] = sum_{kv even/odd} B1p[i', kv] ZT[kv, col]
            # host recombines: out[i'] = Oe+Oo, out[1023-i'] = Oe-Oo.
            # hf outer: the sym halves of ZT are ready before the anti
            # halves (gpsimd recombine lags), so do all hf=0 work first.
            for hf in range(2):
                for io in range(4):
                    last = (hf == 1 and io == 3)
                    pe = pmm.tile([128, 512], f32, name="pe", tag="mm")
                    po = pmm.tile([128, 512], f32, name="po", tag="mm")
                    grps = [(pe, 0, 2), (po, 2, 4)]
                    if last:
                        # odd group first: its vector copy overlaps the even
                        # group's matmuls, shortening the end-of-kernel chain
                        grps = grps[::-1]
                    # second-to-last strip swaps copy engines so the last
                    # strip's scalar copy isn't queued behind it
                    swap = (hf == 1 and io == 2)
                    strip = (hf * 4 + io) * 1024
                    for pt, k0, k1 in grps:
                        for kb in range(k0, k1):
                            o = kb * 512 + io * 128
                            nc.tensor.matmul(
                                pt[:], WB[:, o:o + 128],
                                ZT[s][:, kb * NX + hf * 512:
                                      kb * NX + hf * 512 + 512],
                                start=(kb == k0), stop=(kb == k1 - 1))
                        dst = (Ot[s][:, strip + 512:strip + 1024] if pt is po
                               else Ot[s][:, strip:strip + 512])
                        on_vector = (pt is po) != swap
                        if on_vector:
                            nc.vector.tensor_scalar_mul(dst, pt[:], 1.0)
                        else:
                            nc.scalar.copy(dst, pt[:])
                    r0 = (img * 8 + hf * 4 + io) * 128
                    # last strip: issue from the act ring (idle by then) right
                    # behind its scalar copy; others ride the sync ring
                    eng = nc.scalar if last else nc.sync
                    eng.dma_start(y_d[r0:r0 + 128, :],
                                  Ot[s][:, strip:strip + 1024])

        # startup-critical bytes in global priority order across the two
        # HWDGE rings.  DMA completion sems fire ~2.3us after the last byte
        # (HBM write-receipt), so the first chunks are small to minimize the
        # time to the first released sem; img0 rides the sync ring (starts
        # ~3us before act), img1 + WB ride act.  passA consumes (WA-par0,
        # pos0), (WA-par1, pos1), pos2, pos3 per image, in that order.
        def ldx(eng, s, img, pos, c0=0, c1=2048):
            eng.dma_start(Xq[s][:, pos * 2048 + c0: pos * 2048 + c1],
                          x0_d[img * 128:(img + 1) * 128,
                               pos * 2048 + c0: pos * 2048 + c1])
        # all img0-critical bytes ride the sync ring SOLO: the act ring's
        # X1 issue is gated behind the WA-par0 DMA (tiny scalar copy below),
        # because the SDMA round-robin favors whichever ring has bigger
        # packets and a busy act ring starves the startup-critical chunks
        # ALL input DMAs ride the sync HWDGE ring, in consumption order:
        # the ring drains FIFO at full solo rate (~400 B/ns), so each
        # chunk's completion sem fires right before passA needs it, with no
        # second-ring arbitration and no scheduler-reordering hazards.
        nc.sync.dma_start(WA[:, :512], wa_d[:, :512])         # par0/ci01
        ldx(nc.sync, 0, 0, 0, 0, 1024)                        # pos0/ci01
        nc.sync.dma_start(WA[:, 512:1024], wa_d[:, 512:1024])
        ldx(nc.sync, 0, 0, 0, 1024, 2048)                     # pos0/ci23
        nc.sync.dma_start(WA[:, 1024:], wa_d[:, 1024:])       # par1
        ldx(nc.sync, 0, 0, 1)                                 # pos1
        ldx(nc.sync, 0, 0, 2)                                 # pos2
        ldx(nc.sync, 0, 0, 3)                                 # pos3
        for pos in range(4):
            ldx(nc.sync, 1, 1, pos)                           # img1
        nc.sync.dma_start(LAM[:], lam_d[:, :])
        nc.sync.dma_start(WB[:], wb_d[:, :])

        # C's kg-halves depend on a single pj-phase of A, so they slot in
        # right after it and soak the PE while the next input chunk lands
        passA_pj(0, 0); passC_kg(0, 0)
        passA_pj(0, 1); passC_kg(0, 1)
        passA_pj(1, 0); passC_kg(1, 0)
        passA_pj(1, 1); passC_kg(1, 1)
        passE(0); passE(1)
        passG(0, 0); passG(1, 1)

    nc.compile()
    return nc, _host_weights_k512(n_iter)


def _unfold_output_k512(raw_f16):
    """raw: [8*128, 1024] f16 per image stack -> [IMGS, NX, NX] f32.
    Strip s = hf*4 + io: [Oe(512) | Oo(512)]; rows i' = io*128 + p.
    out[i', j<512] from hf=0, out[i', j>=512] from hf=1 reversed;
    out[i'] = Oe+Oo, out[1023-i'] = Oe-Oo."""
    raw = raw_f16.astype(np.float32).reshape(IMGS_PER_CORE, 2, 4, 128, 1024)
    Oe = raw[..., :512]
    Oo = raw[..., 512:]
    lo = (Oe + Oo).reshape(IMGS_PER_CORE, 2, 512, 512)  # [img, hf, i', j']
    hi = (Oe - Oo).reshape(IMGS_PER_CORE, 2, 512, 512)  # rows 1023-i'
    top = np.concatenate([lo[:, 0], lo[:, 1][..., ::-1]], axis=-1)
    bot = np.concatenate([hi[:, 0], hi[:, 1][..., ::-1]], axis=-1)[:, ::-1]
    return np.concatenate([top, bot], axis=1)


# -------------------------------------------------------------- generic path
# (original implementation; used only when _choose_K(n_iter) != 512)

def _host_weights_gen(n_iter, K):
    A1, B1, lam = _dct_mats()
    kperm = _kperm(K)
    A1t = A1[kperm, :512].T               # [512 (i'/j'), K]   fwd weights
    B1t = B1[:512, :][:, kperm].T         # [K, 512 (j'/i')]   inv weights
    Lam = ((lam[kperm][:, None] + lam[kperm][None, :]) / 4.0) ** n_iter
    KB = K // 128
    WA = A1t.reshape(4, 128, K).transpose(1, 0, 2)
    WA = np.ascontiguousarray(WA.reshape(128, 4 * K)).astype(np.float16)
    WB = B1t.reshape(KB, 128, 512).transpose(1, 0, 2)
    WB = np.ascontiguousarray(WB.reshape(128, KB * 512)).astype(np.float16)
    LAM = Lam.reshape(KB, 128, K).transpose(1, 0, 2)
    LAM = np.ascontiguousarray(LAM.reshape(128, KB * K)).astype(np.float32)
    return {"wa": WA, "wb": WB, "lam": LAM}


def _build_program_gen(n_iter):
    import concourse.bacc as bacc
    import concourse.mybir as mybir
    import concourse.tile as tile

    K = _choose_K(n_iter)
    KB = K // 128          # mode blocks (parity-permuted: KB/2 even, KB/2 odd)
    KH = KB // 2           # blocks per parity
    KP = K // 2            # modes per parity
    nslots = 2 if K <= 512 else 1
    f16 = mybir.dt.float16
    f32 = mybir.dt.float32
    mult = mybir.AluOpType.mult
    add = mybir.AluOpType.add
    sub = mybir.AluOpType.subtract

    nc = bacc.Bacc("TRN2", target_bir_lowering=False, debug=False)
    x0_d = nc.dram_tensor("x0", [IMGS_PER_CORE * 128, 16 * 512], f16,
                          kind="ExternalInput").ap()
    wa_d = nc.dram_tensor("wa", [128, 4 * K], f16, kind="ExternalInput").ap()
    wb_d = nc.dram_tensor("wb", [128, KB * 512], f16,
                          kind="ExternalInput").ap()
    lam_d = nc.dram_tensor("lam", [128, KB * K], f32, kind="ExternalInput").ap()
    y_d = nc.dram_tensor("y", [IMGS_PER_CORE * NX, NX], f16,
                         kind="ExternalOutput").ap()

    with tile.TileContext(nc) as tc, ExitStack() as ctx:
        wp = ctx.enter_context(tc.tile_pool(name="w", bufs=1))
        bp = ctx.enter_context(tc.tile_pool(name="b", bufs=1))
        psum_bufs = 8 if K <= 512 else 4
        pmm = ctx.enter_context(tc.tile_pool(name="pmm", bufs=psum_bufs,
                                             space="PSUM"))
        sp = ctx.enter_context(tc.tile_pool(name="sp", bufs=6))

        WA = wp.tile([128, 4 * K], f16)
        WB = wp.tile([128, KB * 512], f16)
        LAM = wp.tile([128, KB * K], f32)

        Xq = [bp.tile([128, 16 * 512], f16, name=f"x{s}") for s in range(nslots)]
        UTe = [bp.tile([128, 4 * K], f16, name=f"ute{s}") for s in range(nslots)]
        UTo = [bp.tile([128, 4 * K], f16, name=f"uto{s}") for s in range(nslots)]
        WC = [bp.tile([128, KB * K], f16, name=f"wc{s}") for s in range(nslots)]
        ZT = [bp.tile([128, KB * NX], f16, name=f"zt{s}") for s in range(nslots)]
        Ol = [bp.tile([128, 4 * NX], f16, name=f"ol{s}") for s in range(nslots)]
        Oh = [bp.tile([128, 4 * NX], f16, name=f"oh{s}") for s in range(nslots)]

        Wz = bp.tile([128, 512], f16, name="wz")
        nc.gpsimd.memset(Wz[:], 0.0)
        pw = pmm.tile([128, 512], f32, name="pw", tag="mm")
        for r in range(8):
            nc.tensor.matmul(pw[:], Wz[:, :128], Wz[:],
                             start=(r == 0), stop=(r == 7))
        pw2 = pmm.tile([128, 512], f32, name="pw2", tag="mm")
        for r in range(24):
            nc.tensor.matmul(pw2[:, :128], Wz[:, :128], Wz[:, :128],
                             start=(r == 0), stop=(r == 23))

        def load_x(s, img):
            r0 = img * 128
            for h in range(2):           # one DMA per pj half [128, 4096]
                nc.sync.dma_start(Xq[s][:, h * 4096:(h + 1) * 4096],
                                  x0_d[r0:r0 + 128, h * 4096:(h + 1) * 4096])

        def passA(s):
            for pj in range(2):
                for jb in range(4):
                    p = pmm.tile([128, 2 * KP], f32, name="pm", tag="mm")
                    for par in range(2):           # k parity: even, odd
                        pos = 2 * pj + par
                        for ci in range(4):
                            nc.tensor.matmul(
                                p[:, par * KP:(par + 1) * KP],
                                Xq[s][:, (pos * 4 + ci) * 512 + jb * 128:
                                       (pos * 4 + ci) * 512 + jb * 128 + 128],
                                WA[:, ci * K + par * KP: ci * K + (par + 1) * KP],
                                start=(ci == 0), stop=(ci == 3))
                    dst = UTe[s] if pj == 0 else UTo[s]
                    nc.scalar.copy(dst[:, jb * K:(jb + 1) * K], p[:])

        def passC(s):
            for ko in range(KB):
                rhs = UTe[s] if ko < KH else UTo[s]
                for f0 in range(0, K, 512):
                    fw = min(512, K - f0)
                    p = pmm.tile([128, fw], f32, name="pm", tag="mm")
                    for jb in range(4):
                        nc.tensor.matmul(
                            p[:], WA[:, jb * K + ko * 128: jb * K + ko * 128 + 128],
                            rhs[:, jb * K + f0: jb * K + f0 + fw],
                            start=(jb == 0), stop=(jb == 3))
                    nc.vector.tensor_tensor(
                        WC[s][:, ko * K + f0: ko * K + f0 + fw], p[:],
                        LAM[:, ko * K + f0: ko * K + f0 + fw], op=mult)

        def passE(s):
            for kvb in range(KB):
                pe = pmm.tile([128, 512], f32, name="pe", tag="mm")
                po = pmm.tile([128, 512], f32, name="po", tag="mm")
                for kb in range(KH):
                    nc.tensor.matmul(
                        pe[:], WC[s][:, kb * K + kvb * 128: kb * K + kvb * 128 + 128],
                        WB[:, kb * 512:(kb + 1) * 512],
                        start=(kb == 0), stop=(kb == KH - 1))
                for kb in range(KH, KB):
                    nc.tensor.matmul(
                        po[:], WC[s][:, kb * K + kvb * 128: kb * K + kvb * 128 + 128],
                        WB[:, kb * 512:(kb + 1) * 512],
                        start=(kb == KH), stop=(kb == KB - 1))
                ps = sp.tile([128, 512], f16, name="ps", tag="ps")
                nc.scalar.mul(ps[:], po[:], 2.0)
                nc.vector.scalar_tensor_tensor(
                    ZT[s][:, kvb * NX: kvb * NX + 512], ps[:], 0.5, pe[:],
                    op0=mult, op1=add)
                nc.gpsimd.tensor_tensor(
                    ZT[s][:, kvb * NX + 512: (kvb + 1) * NX],
                    ZT[s][:, kvb * NX: kvb * NX + 512], ps[:], op=sub)

        def passG(s, img):
            r0 = img * NX
            for io in range(4):
                for hf in range(2):
                    pe = pmm.tile([128, 512], f32, name="pe", tag="mm")
                    po = pmm.tile([128, 512], f32, name="po", tag="mm")
                    for kb in range(KH):
                        o = kb * 512 + io * 128
                        nc.tensor.matmul(
                            pe[:], WB[:, o:o + 128],
                            ZT[s][:, kb * NX + hf * 512: kb * NX + hf * 512 + 512],
                            start=(kb == 0), stop=(kb == KH - 1))
                    for kb in range(KH, KB):
                        o = kb * 512 + io * 128
                        nc.tensor.matmul(
                            po[:], WB[:, o:o + 128],
                            ZT[s][:, kb * NX + hf * 512: kb * NX + hf * 512 + 512],
                            start=(kb == KH), stop=(kb == KB - 1))
                    c0 = io * NX + hf * 512
                    ps = sp.tile([128, 512], f16, name="ps", tag="ps")
                    nc.scalar.mul(ps[:], po[:], 2.0)
                    nc.vector.scalar_tensor_tensor(
                        Ol[s][:, c0:c0 + 512], ps[:], 0.5, pe[:],
                        op0=mult, op1=add)
                    if s == nslots - 1 and io >= 2:
                        nc.vector.tensor_tensor(
                            Oh[s][:, c0:c0 + 512], Ol[s][:, c0:c0 + 512],
                            ps[:], op=sub)
                    else:
                        nc.gpsimd.tensor_tensor(
                            Oh[s][:, c0:c0 + 512], Ol[s][:, c0:c0 + 512],
                            ps[:], op=sub)
                nc.sync.dma_start(y_d[r0 + io * 128: r0 + (io + 1) * 128, :],
                                  Ol[s][:, io * NX:(io + 1) * NX])
                nc.sync.dma_start(
                    y_d[r0 + 512 + io * 128: r0 + 512 + (io + 1) * 128, :],
                    Oh[s][:, io * NX:(io + 1) * NX])

        nc.scalar.dma_start(WA[:], wa_d[:, :])
        load_x(0, 0)
        nc.scalar.dma_start(WB[:], wb_d[:, :])
        nc.scalar.dma_start(LAM[:], lam_d[:, :])
        if nslots == 2:
            load_x(1, 1)
            passA(0); passA(1)
            passC(0); passC(1)
            passE(0); passE(1)
            passG(0, 0); passG(1, 1)
        else:
            for img in range(IMGS_PER_CORE):
                if img:
                    load_x(0, img)
                passA(0); passC(0); passE(0); passG(0, img)

    nc.compile()
    return nc, _host_weights_gen(n_iter, _choose_K(n_iter))


# ------------------------------------------------------------------- common

def _fold_input(x_f32):
    """[16, NX, NX] f32 -> [16, 128, 8192] f16 parity quadrants in the
    device SBUF layout: col = (pos*4 + ci)*512 + j', partition = i' % 128."""
    lo = x_f32[:, :512, :]
    hi = x_f32[:, 1023:511:-1, :]
    ia = lo + hi    # i-even
    ib = lo - hi    # i-odd
    quad = np.empty((16, 4, 512, 512), np.float32)
    for q, part in ((0, ia), (2, ib)):
        quad[:, q] = part[:, :, :512] + part[:, :, 1023:511:-1]
        quad[:, q + 1] = part[:, :, :512] - part[:, :, 1023:511:-1]
    quad = quad[:, [0, 2, 1, 3]]     # pj-major device order
    # [16, pos, ci*128+p, j'] -> [16, p, pos, ci, j']
    quad = quad.reshape(16, 4, 4, 128, 512).transpose(0, 3, 1, 2, 4)
    return np.ascontiguousarray(quad.reshape(16, 128, 8192)).astype(np.float16)


_PERM = np.r_[0:512, 1023:511:-1]


def _make_in_maps(x_f32, n_iter):
    """x_f32: [16, NX, NX] float32. Returns (nc, in_maps)."""
    if n_iter not in _compiled_cache:
        if _choose_K(n_iter) == 512:
            _compiled_cache[n_iter] = ("k512", *_build_program_k512(n_iter))
        else:
            _compiled_cache[n_iter] = ("gen", *_build_program_gen(n_iter))
    kind, nc, wdict = _compiled_cache[n_iter]
    xq = _fold_input(x_f32)
    in_maps = []
    for c in range(N_CORES):
        shard = np.ascontiguousarray(
            xq[c * IMGS_PER_CORE:(c + 1) * IMGS_PER_CORE].reshape(
                IMGS_PER_CORE * 128, 16 * 512))
        m = {"x0": shard}
        m.update(wdict)
        in_maps.append(m)
    return nc, in_maps


def kernel(layout, heat, n_iter):
    n_iter = int(n_iter)
    heat = np.asarray(heat)
    out_shape = heat.shape
    x = np.asarray(heat, np.float32).reshape(16, NX, NX)
    if n_iter <= 0:
        return x.reshape(out_shape).copy()

    from concourse.bass_utils import run_bass_kernel_spmd

    nc, in_maps = _make_in_maps(x, n_iter)
    kind = _compiled_cache[n_iter][0]
    res = run_bass_kernel_spmd(nc, in_maps, core_ids=list(range(N_CORES)))
    out = np.empty((16, NX, NX), np.float32)
    for c in range(N_CORES):
        raw = res.results[c]["y"]
        if kind == "k512":
            out[c * IMGS_PER_CORE:(c + 1) * IMGS_PER_CORE] = (
                _unfold_output_k512(raw))
        else:
            r = raw.astype(np.float32).reshape(IMGS_PER_CORE, NX, NX)
            out[c * IMGS_PER_CORE:(c + 1) * IMGS_PER_CORE] = (
                r[:, _PERM][:, :, _PERM])
    return out.reshape(out_shape)


# revision 24
# speedup vs baseline: 1.1407x; 1.1407x over previous
"""Trainium2 Bass kernel for n-iteration Jacobi (3x3 cross stencil, reflect pad).

x_{t+1} = 0.25*(V + H) x_t + f,  f = COF*layout (|f| ~ 2.4e-9, contributes
< 3e-6 relative to the output; dropped).

V (vertical) and H (horizontal) neighbor-sum operators with this reflect
boundary are exactly diagonalized by the DCT-I basis v_k[i] = cos(pi*i*k/1023),
eigenvalues lam_k = 2*cos(pi*k/1023).  n Jacobi iterations collapse to one
spectral sandwich per image:

    out = C_k @ (Lam2D * (Cinv_k @ X @ Cinv_k^T)) @ C_k^T
    Lam2D[a,b] = ((lam_a + lam_b)/4)^n

Reductions on top of the plain sandwich:
  1. Mode truncation: keep K=512 of 1024 modes per axis for n=50 (256 lowest
     + 256 highest; max truncated |Lam| ~ 4e-4).
  2. Even/odd folding: cos(pi*k*(1023-i)/1023) = (-1)^k cos(pi*k*i/1023),
     so folding the spatial axes into symmetric/antisymmetric halves halves
     every contraction.  Input fold on the host; output parity recombination
     ALSO on the host (kernel emits the even/odd partial sums Oe/Oo).
  3. Corner sparsity: Lam2D is non-negligible only for same-corner mode
     pairs (low-low near (0,0), high-high near (pi,pi)); cross terms are
     <= 0.147^n ~ 0 for n>=30.  The mode-space passes (C: forward-horizontal
     + Lam, E: inverse-horizontal) contract only same-corner blocks, halving
     both.  Enabled by a corner-major mode layout in UT (scatter copy-out
     from passA's PSUM).
  4. No PE transposes: passes needing transposed outputs run with the data
     as the stationary lhsT operand.
  5. DMA: the two HWDGE rings (sync, act) carry the startup-critical bytes
     in priority order so passA can start as soon as ~1.5 MiB has landed.

Per image: 120 matmuls (A:64x256c, C:16x256c, E:8x512c, G:32x512c) ~ 41K PE
rows at 1 row/cycle.  All matmul operands fp16 (PSUM accumulates fp32).
Per core: 2 of 16 images, passes software-pipelined across the two images.
"""

import math
from contextlib import ExitStack

import numpy as np

NX = 1024
N_CORES = 8
IMGS_PER_CORE = 2
LN_TAU = math.log(1e4)

_compiled_cache = {}


def _choose_K(n_iter):
    # keep modes with ((lam_a+lam_b)/4)^n >= 1e-4; parity folding needs
    # K to be a multiple of 256
    R = int(math.ceil(1023.0 / math.pi * math.sqrt(2.0 * LN_TAU / max(n_iter, 1))))
    K = min(1024, ((2 * R + 255) // 256) * 256)
    return K


def _dct_mats():
    i = np.arange(NX)
    C = np.cos(np.pi * np.outer(i, i) / (NX - 1))
    lam = 2.0 * np.cos(np.pi * i / (NX - 1))
    w = np.ones(NX)
    w[0] = w[-1] = 0.5
    s = math.sqrt(2.0 / (NX - 1))
    # C^{-1} = (2/(N-1)) W C W; balance fp16 range: A1 = Cinv/s, B1 = C*s
    A1 = (2.0 / (NX - 1) / s) * (w[:, None] * C * w[None, :])
    B1 = C * s
    return A1, B1, lam


def _kperm(K):
    R = K // 2
    kept = np.r_[0:R, NX - R:NX]
    return np.r_[kept[kept % 2 == 0], kept[kept % 2 == 1]]  # evens, then odds


# ---------------------------------------------------------------- fast path
# K=512 only.  kperm blocks (128 modes each): 0=low-even, 1=high-even,
# 2=low-odd, 3=high-odd.  corner(block b) = b % 2 (0=low, 1=high).

def _host_weights_k512(n_iter):
    K = 512
    A1, B1, lam = _dct_mats()
    kperm = _kperm(K)
    A1t = A1[kperm, :512].T               # [512 (i'/j'), K]   fwd weights
    B1t = B1[:512, :][:, kperm].T         # [K, 512 (j'/i')]   inv weights
    # WA par-major: WA[c, par*1024 + ci*256 + m] = A1t[ci*128 + c,
    # par*256 + m] so the par-0 half is a 0.25 MiB startup-critical DMA
    WA0 = A1t.reshape(4, 128, K).transpose(1, 0, 2).reshape(128, 4, 2, 256)
    WA = np.ascontiguousarray(
        WA0.transpose(0, 2, 1, 3).reshape(128, 4 * K)).astype(np.float16)
    # WB[c, kblk*512 + f] = B1t[kblk*128 + c, f]  (kblk: mode block)
    WB = B1t.reshape(4, 128, 512).transpose(1, 0, 2)
    WB = np.ascontiguousarray(WB.reshape(128, 4 * 512)).astype(np.float16)
    # LAM2[c, ko*256 + q*128 + m] = Lam(kperm[ko*128+c], kperm[q*256 +
    # corner(ko)*128 + m]) -- same-corner (kh, kv) pairs only, kv parity q
    Lam = ((lam[kperm][:, None] + lam[kperm][None, :]) / 4.0) ** n_iter
    LAM = np.empty((128, 4 * 256), np.float32)
    for ko in range(4):
        cor = ko % 2
        for q in range(2):
            kv_cols = np.arange(q * 256 + cor * 128, q * 256 + cor * 128 + 128)
            LAM[:, ko * 256 + q * 128: ko * 256 + (q + 1) * 128] = (
                Lam[ko * 128:(ko + 1) * 128][:, kv_cols])
    return {"wa": WA, "wb": WB, "lam": LAM}


def _build_program_k512(n_iter):
    import concourse.bacc as bacc
    import concourse.mybir as mybir
    import concourse.tile as tile

    K = 512
    f16 = mybir.dt.float16
    f32 = mybir.dt.float32
    mult = mybir.AluOpType.mult
    add = mybir.AluOpType.add
    sub = mybir.AluOpType.subtract

    nc = bacc.Bacc("TRN2", target_bir_lowering=False, debug=False)
    # x0: per image the exact SBUF layout [128, 16*512] (quadrant pos, block
    # ci at cols (pos*4+ci)*512); shape-preserving DMAs only
    x0_d = nc.dram_tensor("x0", [IMGS_PER_CORE * 128, 16 * 512], f16,
                          kind="ExternalInput").ap()
    wa_d = nc.dram_tensor("wa", [128, 4 * K], f16, kind="ExternalInput").ap()
    wb_d = nc.dram_tensor("wb", [128, 4 * 512], f16,
                          kind="ExternalInput").ap()
    lam_d = nc.dram_tensor("lam", [128, 4 * 256], f32, kind="ExternalInput").ap()
    # y: per (img, io, hf) strip of 128 rows: [Oe(512) | Oo(512)]; host
    # recombines parities and unfolds
    y_d = nc.dram_tensor("y", [IMGS_PER_CORE * 8 * 128, NX], f16,
                         kind="ExternalOutput").ap()

    with tile.TileContext(nc) as tc, ExitStack() as ctx:
        wp = ctx.enter_context(tc.tile_pool(name="w", bufs=1))
        bp = ctx.enter_context(tc.tile_pool(name="b", bufs=1))
        pmm = ctx.enter_context(tc.tile_pool(name="pmm", bufs=8, space="PSUM"))
        sp = ctx.enter_context(tc.tile_pool(name="sp", bufs=6))

        WA = wp.tile([128, 4 * K], f16)
        WB = wp.tile([128, 4 * 512], f16)
        LAM = wp.tile([128, 4 * 256], f32)

        # Xq: 16 blocks of [128, 512]: pos = 2*pj + par (pj: j-fold parity,
        # par: i-fold parity), block = pos*4 + ci
        Xq = [bp.tile([128, 16 * 512], f16, name=f"x{s}") for s in range(2)]
        # UT_pj[j', kv], corner-major within each jb block of 512:
        # col = jb*512 + corner*256 + par*128 + c
        UTe = [bp.tile([128, 4 * K], f16, name=f"ute{s}") for s in range(2)]
        UTo = [bp.tile([128, 4 * K], f16, name=f"uto{s}") for s in range(2)]
        # WC[kh, kv]*Lam: col = ko*256 + kvpar*128 + m (kv same corner as ko)
        WC = [bp.tile([128, 4 * 256], f16, name=f"wc{s}") for s in range(2)]
        # ZT[kv, col]: strip kvb at col kvb*1024: [sym j' 512 | anti j' 512]
        ZT = [bp.tile([128, 4 * NX], f16, name=f"zt{s}") for s in range(2)]
        # O[(io*2+hf)*1024 + [Oe 512 | Oo 512]] -- even/odd kv partial sums
        Ot = [bp.tile([128, 8 * NX // 1], f16, name=f"ot{s}") for s in range(2)]

        # PE warmup: ramp the tensor engine's pstate on zeros while the
        # first input/weight DMAs are still in flight
        Wz = bp.tile([128, 512], f16, name="wz")
        nc.gpsimd.memset(Wz[:], 0.0)
        pw = pmm.tile([128, 512], f32, name="pw", tag="mm")
        for r in range(8):
            nc.tensor.matmul(pw[:], Wz[:, :128], Wz[:],
                             start=(r == 0), stop=(r == 7))
        # fine-grained filler so the queue can drain the moment data lands
        pw2 = pmm.tile([128, 512], f32, name="pw2", tag="mm")
        for r in range(8):
            nc.tensor.matmul(pw2[:, :128], Wz[:, :128], Wz[:, :128],
                             start=(r == 0), stop=(r == 7))

        def passA_pj(s, pj):
            # UT_pj[j', k] = sum_{i'} Xq[pos][i', j'] * A1t[i', k]
            # lhsT = input quadrant block, rhs = WA par-slice; the PSUM is
            # par-major [par: low|high]; the copy-out scatters to the
            # corner-major UT layout [cor: par0|par1].  All par-0 groups are
            # emitted before any par-1 group so the PE consumes data in DMA
            # arrival order.
            dst = UTe[s] if pj == 0 else UTo[s]
            ptiles = []
            for jb in range(4):
                p = pmm.tile([128, 512], f32, name="pm", tag="mm")
                ptiles.append(p)
                pos = 2 * pj
                for ci in range(4):
                    nc.tensor.matmul(
                        p[:, 0:256],
                        Xq[s][:, (pos * 4 + ci) * 512 + jb * 128:
                               (pos * 4 + ci) * 512 + jb * 128 + 128],
                        WA[:, ci * 256: (ci + 1) * 256],
                        start=(ci == 0), stop=(ci == 3))
            for jb in range(4):
                p = ptiles[jb]
                pos = 2 * pj + 1
                for ci in range(4):
                    nc.tensor.matmul(
                        p[:, 256:512],
                        Xq[s][:, (pos * 4 + ci) * 512 + jb * 128:
                               (pos * 4 + ci) * 512 + jb * 128 + 128],
                        WA[:, 1024 + ci * 256: 1024 + (ci + 1) * 256],
                        start=(ci == 0), stop=(ci == 3))
                src = p[:].rearrange("p (par cor c) -> p par cor c",
                                     par=2, cor=2, c=128)
                out = dst[:, jb * 512:(jb + 1) * 512].rearrange(
                    "p (cor par c) -> p par cor c", cor=2, par=2, c=128)
                if (pj * 4 + jb) % 2 == 0:
                    nc.scalar.copy(out, src)
                else:
                    nc.vector.tensor_scalar_mul(out, src, 1.0)

        def passC_kg(s, kg):
            # WC[kh, kv] = Lam * sum_{j'} A1p[kh, j'] UT_{par(kh)}[j', kv]
            # same-corner kv only (cross-corner Lam^n ~ 0).  kg=0 (even kh)
            # depends only on UTe (pj=0), kg=1 only on UTo -- interleaved
            # between passA pj-phases to absorb input-DMA arrival gaps.
            p = pmm.tile([128, 512], f32, name="pm", tag="mm")
            for half in range(2):
                ko = kg * 2 + half          # kperm block: 0=LE 1=HE 2=LO 3=HO
                cor = ko % 2
                kpar = ko // 2              # WA par-major half
                rhs_src = UTe[s] if ko < 2 else UTo[s]
                for jb in range(4):
                    nc.tensor.matmul(
                        p[:, half * 256:(half + 1) * 256],
                        WA[:, kpar * 1024 + jb * 256 + cor * 128:
                            kpar * 1024 + jb * 256 + cor * 128 + 128],
                        rhs_src[:, jb * 512 + cor * 256:
                                jb * 512 + cor * 256 + 256],
                        start=(jb == 0), stop=(jb == 3))
            nc.vector.tensor_tensor(
                WC[s][:, kg * 512:(kg + 1) * 512], p[:],
                LAM[:, kg * 512:(kg + 1) * 512], op=mult)

        def passE(s):
            # ZeT/ZoT[kv, j'] = sum_{kh even/odd, same corner} WC[kh, kv]
            #                   * B1t[kh, j']
            # ZT strip: sym = Ze+Zo (Z at j'), anti = Ze-Zo (Z at 1023-j')
            for kvb in range(4):            # kv block: 0=LE 1=HE 2=LO 3=HO
                cor = kvb % 2
                kvpar = kvb // 2            # 0 = even kv chunk, 1 = odd
                ko_e = cor                  # even-kh block, same corner
                ko_o = 2 + cor              # odd-kh block, same corner
                pe = pmm.tile([128, 512], f32, name="pe", tag="mm")
                po = pmm.tile([128, 512], f32, name="po", tag="mm")
                nc.tensor.matmul(
                    pe[:], WC[s][:, ko_e * 256 + kvpar * 128:
                                 ko_e * 256 + kvpar * 128 + 128],
                    WB[:, ko_e * 512:(ko_e + 1) * 512], start=True, stop=True)
                nc.tensor.matmul(
                    po[:], WC[s][:, ko_o * 256 + kvpar * 128:
                                 ko_o * 256 + kvpar * 128 + 128],
                    WB[:, ko_o * 512:(ko_o + 1) * 512], start=True, stop=True)
                ps = sp.tile([128, 512], f16, name="ps", tag="ps")
                nc.scalar.mul(ps[:], po[:], 2.0)
                nc.vector.scalar_tensor_tensor(
                    ZT[s][:, kvb * NX: kvb * NX + 512], ps[:], 0.5, pe[:],
                    op0=mult, op1=add)
                nc.gpsimd.tensor_tensor(
                    ZT[s][:, kvb * NX + 512: (kvb + 1) * NX],
                    ZT[s][:, kvb * NX: kvb * NX + 512], ps[:], op=sub)

        def passG(s, img):
            # Oe/Oo[i', col] = sum_{kv even/odd} B1p[i', kv] ZT[kv, col]
            # host recombines: out[i'] = Oe+Oo, out[1023-i'] = Oe-Oo.
            # hf outer: the sym halves of ZT are ready before the anti
            # halves (gpsimd recombine lags), so do all hf=0 work first.
            for hf in range(2):
                for io in range(4):
                    last = (hf == 1 and io == 3)
                    pe = pmm.tile([128, 512], f32, name="pe", tag="mm")
                    po = pmm.tile([128, 512], f32, name="po", tag="mm")
                    grps = [(pe, 0, 2), (po, 2, 4)]
                    if last:
                        # odd group first: its vector copy overlaps the even
                        # group's matmuls, shortening the end-of-kernel chain
                        grps = grps[::-1]
                    strip = (hf * 4 + io) * 1024
                    for pt, k0, k1 in grps:
                        for kb in range(k0, k1):
                            o = kb * 512 + io * 128
                            nc.tensor.matmul(
                                pt[:], WB[:, o:o + 128],
                                ZT[s][:, kb * NX + hf * 512:
                                      kb * NX + hf * 512 + 512],
                                start=(kb == k0), stop=(kb == k1 - 1))
                        if pt is po:
                            nc.vector.tensor_scalar_mul(
                                Ot[s][:, strip + 512:strip + 1024], pt[:], 1.0)
                        else:
                            nc.scalar.copy(Ot[s][:, strip:strip + 512], pt[:])
                    r0 = (img * 8 + hf * 4 + io) * 128
                    # last strip: issue from the act ring (idle by then) right
                    # behind its scalar copy; others ride the sync ring
                    eng = nc.scalar if last else nc.sync
                    eng.dma_start(y_d[r0:r0 + 128, :],
                                  Ot[s][:, strip:strip + 1024])

        # startup-critical bytes in global priority order across the two
        # HWDGE rings.  DMA completion sems fire ~2.3us after the last byte
        # (HBM write-receipt), so the first chunks are small to minimize the
        # time to the first released sem; img0 rides the sync ring (starts
        # ~3us before act), img1 + WB ride act.  passA consumes (WA-par0,
        # pos0), (WA-par1, pos1), pos2, pos3 per image, in that order.
        def ldx(eng, s, img, pos, c0=0, c1=2048):
            eng.dma_start(Xq[s][:, pos * 2048 + c0: pos * 2048 + c1],
                          x0_d[img * 128:(img + 1) * 128,
                               pos * 2048 + c0: pos * 2048 + c1])
        # all img0-critical bytes ride the sync ring SOLO: the act ring's
        # X1 issue is gated behind the WA-par0 DMA (tiny scalar copy below),
        # because the SDMA round-robin favors whichever ring has bigger
        # packets and a busy act ring starves the startup-critical chunks
        # ALL input DMAs ride the sync HWDGE ring, in consumption order:
        # the ring drains FIFO at full solo rate (~400 B/ns), so each
        # chunk's completion sem fires right before passA needs it, with no
        # second-ring arbitration and no scheduler-reordering hazards.
        nc.sync.dma_start(WA[:, :512], wa_d[:, :512])         # par0/ci01
        ldx(nc.sync, 0, 0, 0, 0, 1024)                        # pos0/ci01
        nc.sync.dma_start(WA[:, 512:1024], wa_d[:, 512:1024])
        ldx(nc.sync, 0, 0, 0, 1024, 2048)                     # pos0/ci23
        nc.sync.dma_start(WA[:, 1024:], wa_d[:, 1024:])       # par1
        ldx(nc.sync, 0, 0, 1)                                 # pos1
        ldx(nc.sync, 0, 0, 2)                                 # pos2
        ldx(nc.sync, 0, 0, 3)                                 # pos3
        for pos in range(4):
            ldx(nc.sync, 1, 1, pos)                           # img1
        nc.sync.dma_start(LAM[:], lam_d[:, :])
        nc.sync.dma_start(WB[:], wb_d[:, :])

        # C's kg-halves depend on a single pj-phase of A, so they slot in
        # right after it and soak the PE while the next input chunk lands
        passA_pj(0, 0); passC_kg(0, 0)
        passA_pj(0, 1); passC_kg(0, 1)
        passA_pj(1, 0); passC_kg(1, 0)
        passA_pj(1, 1); passC_kg(1, 1)
        passE(0); passE(1)
        passG(0, 0); passG(1, 1)

    nc.compile()
    return nc, _host_weights_k512(n_iter)


def _unfold_output_k512(raw_f16):
    """raw: [8*128, 1024] f16 per image stack -> [IMGS, NX, NX] f32.
    Strip s = hf*4 + io: [Oe(512) | Oo(512)]; rows i' = io*128 + p.
    out[i', j<512] from hf=0, out[i', j>=512] from hf=1 reversed;
    out[i'] = Oe+Oo, out[1023-i'] = Oe-Oo."""
    raw = raw_f16.astype(np.float32).reshape(IMGS_PER_CORE, 2, 4, 128, 1024)
    Oe = raw[..., :512]
    Oo = raw[..., 512:]
    lo = (Oe + Oo).reshape(IMGS_PER_CORE, 2, 512, 512)  # [img, hf, i', j']
    hi = (Oe - Oo).reshape(IMGS_PER_CORE, 2, 512, 512)  # rows 1023-i'
    top = np.concatenate([lo[:, 0], lo[:, 1][..., ::-1]], axis=-1)
    bot = np.concatenate([hi[:, 0], hi[:, 1][..., ::-1]], axis=-1)[:, ::-1]
    return np.concatenate([top, bot], axis=1)


# -------------------------------------------------------------- generic path
# (original implementation; used only when _choose_K(n_iter) != 512)

def _host_weights_gen(n_iter, K):
    A1, B1, lam = _dct_mats()
    kperm = _kperm(K)
    A1t = A1[kperm, :512].T               # [512 (i'/j'), K]   fwd weights
    B1t = B1[:512, :][:, kperm].T         # [K, 512 (j'/i')]   inv weights
    Lam = ((lam[kperm][:, None] + lam[kperm][None, :]) / 4.0) ** n_iter
    KB = K // 128
    WA = A1t.reshape(4, 128, K).transpose(1, 0, 2)
    WA = np.ascontiguousarray(WA.reshape(128, 4 * K)).astype(np.float16)
    WB = B1t.reshape(KB, 128, 512).transpose(1, 0, 2)
    WB = np.ascontiguousarray(WB.reshape(128, KB * 512)).astype(np.float16)
    LAM = Lam.reshape(KB, 128, K).transpose(1, 0, 2)
    LAM = np.ascontiguousarray(LAM.reshape(128, KB * K)).astype(np.float32)
    return {"wa": WA, "wb": WB, "lam": LAM}


def _build_program_gen(n_iter):
    import concourse.bacc as bacc
    import concourse.mybir as mybir
    import concourse.tile as tile

    K = _choose_K(n_iter)
    KB = K // 128          # mode blocks (parity-permuted: KB/2 even, KB/2 odd)
    KH = KB // 2           # blocks per parity
    KP = K // 2            # modes per parity
    nslots = 2 if K <= 512 else 1
    f16 = mybir.dt.float16
    f32 = mybir.dt.float32
    mult = mybir.AluOpType.mult
    add = mybir.AluOpType.add
    sub = mybir.AluOpType.subtract

    nc = bacc.Bacc("TRN2", target_bir_lowering=False, debug=False)
    x0_d = nc.dram_tensor("x0", [IMGS_PER_CORE * 128, 16 * 512], f16,
                          kind="ExternalInput").ap()
    wa_d = nc.dram_tensor("wa", [128, 4 * K], f16, kind="ExternalInput").ap()
    wb_d = nc.dram_tensor("wb", [128, KB * 512], f16,
                          kind="ExternalInput").ap()
    lam_d = nc.dram_tensor("lam", [128, KB * K], f32, kind="ExternalInput").ap()
    y_d = nc.dram_tensor("y", [IMGS_PER_CORE * NX, NX], f16,
                         kind="ExternalOutput").ap()

    with tile.TileContext(nc) as tc, ExitStack() as ctx:
        wp = ctx.enter_context(tc.tile_pool(name="w", bufs=1))
        bp = ctx.enter_context(tc.tile_pool(name="b", bufs=1))
        psum_bufs = 8 if K <= 512 else 4
        pmm = ctx.enter_context(tc.tile_pool(name="pmm", bufs=psum_bufs,
                                             space="PSUM"))
        sp = ctx.enter_context(tc.tile_pool(name="sp", bufs=6))

        WA = wp.tile([128, 4 * K], f16)
        WB = wp.tile([128, KB * 512], f16)
        LAM = wp.tile([128, KB * K], f32)

        Xq = [bp.tile([128, 16 * 512], f16, name=f"x{s}") for s in range(nslots)]
        UTe = [bp.tile([128, 4 * K], f16, name=f"ute{s}") for s in range(nslots)]
        UTo = [bp.tile([128, 4 * K], f16, name=f"uto{s}") for s in range(nslots)]
        WC = [bp.tile([128, KB * K], f16, name=f"wc{s}") for s in range(nslots)]
        ZT = [bp.tile([128, KB * NX], f16, name=f"zt{s}") for s in range(nslots)]
        Ol = [bp.tile([128, 4 * NX], f16, name=f"ol{s}") for s in range(nslots)]
        Oh = [bp.tile([128, 4 * NX], f16, name=f"oh{s}") for s in range(nslots)]

        Wz = bp.tile([128, 512], f16, name="wz")
        nc.gpsimd.memset(Wz[:], 0.0)
        pw = pmm.tile([128, 512], f32, name="pw", tag="mm")
        for r in range(8):
            nc.tensor.matmul(pw[:], Wz[:, :128], Wz[:],
                             start=(r == 0), stop=(r == 7))
        pw2 = pmm.tile([128, 512], f32, name="pw2", tag="mm")
        for r in range(24):
            nc.tensor.matmul(pw2[:, :128], Wz[:, :128], Wz[:, :128],
                             start=(r == 0), stop=(r == 23))

        def load_x(s, img):
            r0 = img * 128
            for h in range(2):           # one DMA per pj half [128, 4096]
                nc.sync.dma_start(Xq[s][:, h * 4096:(h + 1) * 4096],
                                  x0_d[r0:r0 + 128, h * 4096:(h + 1) * 4096])

        def passA(s):
            for pj in range(2):
                for jb in range(4):
                    p = pmm.tile([128, 2 * KP], f32, name="pm", tag="mm")
                    for par in range(2):           # k parity: even, odd
                        pos = 2 * pj + par
                        for ci in range(4):
                            nc.tensor.matmul(
                                p[:, par * KP:(par + 1) * KP],
                                Xq[s][:, (pos * 4 + ci) * 512 + jb * 128:
                                       (pos * 4 + ci) * 512 + jb * 128 + 128],
                                WA[:, ci * K + par * KP: ci * K + (par + 1) * KP],
                                start=(ci == 0), stop=(ci == 3))
                    dst = UTe[s] if pj == 0 else UTo[s]
                    nc.scalar.copy(dst[:, jb * K:(jb + 1) * K], p[:])

        def passC(s):
            for ko in range(KB):
                rhs = UTe[s] if ko < KH else UTo[s]
                for f0 in range(0, K, 512):
                    fw = min(512, K - f0)
                    p = pmm.tile([128, fw], f32, name="pm", tag="mm")
                    for jb in range(4):
                        nc.tensor.matmul(
                            p[:], WA[:, jb * K + ko * 128: jb * K + ko * 128 + 128],
                            rhs[:, jb * K + f0: jb * K + f0 + fw],
                            start=(jb == 0), stop=(jb == 3))
                    nc.vector.tensor_tensor(
                        WC[s][:, ko * K + f0: ko * K + f0 + fw], p[:],
                        LAM[:, ko * K + f0: ko * K + f0 + fw], op=mult)

        def passE(s):
            for kvb in range(KB):
                pe = pmm.tile([128, 512], f32, name="pe", tag="mm")
                po = pmm.tile([128, 512], f32, name="po", tag="mm")
                for kb in range(KH):
                    nc.tensor.matmul(
                        pe[:], WC[s][:, kb * K + kvb * 128: kb * K + kvb * 128 + 128],
                        WB[:, kb * 512:(kb + 1) * 512],
                        start=(kb == 0), stop=(kb == KH - 1))
                for kb in range(KH, KB):
                    nc.tensor.matmul(
                        po[:], WC[s][:, kb * K + kvb * 128: kb * K + kvb * 128 + 128],
                        WB[:, kb * 512:(kb + 1) * 512],
                        start=(kb == KH), stop=(kb == KB - 1))
                ps = sp.tile([128, 512], f16, name="ps", tag="ps")
                nc.scalar.mul(ps[:], po[:], 2.0)
                nc.vector.scalar_tensor_tensor(
                    ZT[s][:, kvb * NX: kvb * NX + 512], ps[:], 0.5, pe[:],
                    op0=mult, op1=add)
                nc.gpsimd.tensor_tensor(
                    ZT[s][:, kvb * NX + 512: (kvb + 1) * NX],
                    ZT[s][:, kvb * NX: kvb * NX + 512], ps[:], op=sub)

        def passG(s, img):
            r0 = img * NX
            for io in range(4):
                for hf in range(2):
                    pe = pmm.tile([128, 512], f32, name="pe", tag="mm")
                    po = pmm.tile([128, 512], f32, name="po", tag="mm")
                    for kb in range(KH):
                        o = kb * 512 + io * 128
                        nc.tensor.matmul(
                            pe[:], WB[:, o:o + 128],
                            ZT[s][:, kb * NX + hf * 512: kb * NX + hf * 512 + 512],
                            start=(kb == 0), stop=(kb == KH - 1))
                    for kb in range(KH, KB):
                        o = kb * 512 + io * 128
                        nc.tensor.matmul(
                            po[:], WB[:, o:o + 128],
                            ZT[s][:, kb * NX + hf * 512: kb * NX + hf * 512 + 512],
                            start=(kb == KH), stop=(kb == KB - 1))
                    c0 = io * NX + hf * 512
                    ps = sp.tile([128, 512], f16, name="ps", tag="ps")
                    nc.scalar.mul(ps[:], po[:], 2.0)
                    nc.vector.scalar_tensor_tensor(
                        Ol[s][:, c0:c0 + 512], ps[:], 0.5, pe[:],
                        op0=mult, op1=add)
                    if s == nslots - 1 and io >= 2:
                        nc.vector.tensor_tensor(
                            Oh[s][:, c0:c0 + 512], Ol[s][:, c0:c0 + 512],
                            ps[:], op=sub)
                    else:
                        nc.gpsimd.tensor_tensor(
                            Oh[s][:, c0:c0 + 512], Ol[s][:, c0:c0 + 512],
                            ps[:], op=sub)
                nc.sync.dma_start(y_d[r0 + io * 128: r0 + (io + 1) * 128, :],
                                  Ol[s][:, io * NX:(io + 1) * NX])
                nc.sync.dma_start(
                    y_d[r0 + 512 + io * 128: r0 + 512 + (io + 1) * 128, :],
                    Oh[s][:, io * NX:(io + 1) * NX])

        nc.scalar.dma_start(WA[:], wa_d[:, :])
        load_x(0, 0)
        nc.scalar.dma_start(WB[:], wb_d[:, :])
        nc.scalar.dma_start(LAM[:], lam_d[:, :])
        if nslots == 2:
            load_x(1, 1)
            passA(0); passA(1)
            passC(0); passC(1)
            passE(0); passE(1)
            passG(0, 0); passG(1, 1)
        else:
            for img in range(IMGS_PER_CORE):
                if img:
                    load_x(0, img)
                passA(0); passC(0); passE(0); passG(0, img)

    nc.compile()
    return nc, _host_weights_gen(n_iter, _choose_K(n_iter))


# ------------------------------------------------------------------- common

def _fold_input(x_f32):
    """[16, NX, NX] f32 -> [16, 128, 8192] f16 parity quadrants in the
    device SBUF layout: col = (pos*4 + ci)*512 + j', partition = i' % 128."""
    lo = x_f32[:, :512, :]
    hi = x_f32[:, 1023:511:-1, :]
    ia = lo + hi    # i-even
    ib = lo - hi    # i-odd
    quad = np.empty((16, 4, 512, 512), np.float32)
    for q, part in ((0, ia), (2, ib)):
        quad[:, q] = part[:, :, :512] + part[:, :, 1023:511:-1]
        quad[:, q + 1] = part[:, :, :512] - part[:, :, 1023:511:-1]
    quad = quad[:, [0, 2, 1, 3]]     # pj-major device order
    # [16, pos, ci*128+p, j'] -> [16, p, pos, ci, j']
    quad = quad.reshape(16, 4, 4, 128, 512).transpose(0, 3, 1, 2, 4)
    return np.ascontiguousarray(quad.reshape(16, 128, 8192)).astype(np.float16)


_PERM = np.r_[0:512, 1023:511:-1]


def _make_in_maps(x_f32, n_iter):
    """x_f32: [16, NX, NX] float32. Returns (nc, in_maps)."""
    if n_iter not in _compiled_cache:
        if _choose_K(n_iter) == 512:
            _compiled_cache[n_iter] = ("k512", *_build_program_k512(n_iter))
        else:
            _compiled_cache[n_iter] = ("gen", *_build_program_gen(n_iter))
    kind, nc, wdict = _compiled_cache[n_iter]
    xq = _fold_input(x_f32)
    in_maps = []
    for c in range(N_CORES):
        shard = np.ascontiguousarray(
            xq[c * IMGS_PER_CORE:(c + 1) * IMGS_PER_CORE].reshape(
                IMGS_PER_CORE * 128, 16 * 512))
        m = {"x0": shard}
        m.update(wdict)
        in_maps.append(m)
    return nc, in_maps


def kernel(layout, heat, n_iter):
    n_iter = int(n_iter)
    heat = np.asarray(heat)
    out_shape = heat.shape
    x = np.asarray(heat, np.float32).reshape(16, NX, NX)
    if n_iter <= 0:
        return x.reshape(out_shape).copy()

    from concourse.bass_utils import run_bass_kernel_spmd

    nc, in_maps = _make_in_maps(x, n_iter)
    kind = _compiled_cache[n_iter][0]
    res = run_bass_kernel_spmd(nc, in_maps, core_ids=list(range(N_CORES)))
    out = np.empty((16, NX, NX), np.float32)
    for c in range(N_CORES):
        raw = res.results[c]["y"]
        if kind == "k512":
            out[c * IMGS_PER_CORE:(c + 1) * IMGS_PER_CORE] = (
                _unfold_output_k512(raw))
        else:
            r = raw.astype(np.float32).reshape(IMGS_PER_CORE, NX, NX)
            out[c * IMGS_PER_CORE:(c + 1) * IMGS_PER_CORE] = (
                r[:, _PERM][:, :, _PERM])
    return out.reshape(out_shape)
